# revision 16
# baseline (speedup 1.0000x reference)
"""MinGRU block kernel v4 for Trainium2 (Bass/Tile), SPMD over 8 NeuronCores.

Problem: B=8, S=2048, D=1024, F=3072. Data-parallel: one batch row per core.

Host-side transposes (x arrives as f16 xT [KD,128,S], output leaves as f16
outT [MD,128,S]); fused per-chunk software pipeline; out1 residual resident in
SBUF f16; norm squares on DVE + DoubleRow ones-reduce on PE; per-engine
instruction streams are emission-ordered to avoid ACT table thrash (sigma /
tanh / silu / sqrt grouped; 4 table switches per chunk) and to keep the PE
matmul stream dense (reduce-MMs placed after the FFN gate/up block so the
DVE scan chain is long since done).

Scales (as v2): acts fp8 with AS=8 folded into rms reciprocal; mixer weights
x S_MIX, gate x S_GU, up x S_UP, out x S_O; z carries ZETA = AS*S_UP = 32;
final readout descales by 1/(ZETA*S_O).
"""

import os
import sys
from contextlib import ExitStack

import numpy as np
import ml_dtypes

for _p in ("/opt/trn_rl_repo", "/root/.axon_site/_ro/trn_rl_repo"):
    if os.path.isdir(_p) and _p not in sys.path:
        sys.path.insert(0, _p)

import concourse.bass as bass
import concourse.tile as tile
from concourse import bacc, mybir
from concourse.bass_utils import run_bass_kernel_spmd

F32 = mybir.dt.float32
F16 = mybir.dt.float16
F8 = mybir.dt.float8e4
AF = mybir.ActivationFunctionType
OP = mybir.AluOpType
DR = mybir.MatmulPerfMode.DoubleRow

B, S, D, F = 8, 2048, 1024, 3072
EPS = 1e-6
KD = D // 128            # 8
MD = D // 128            # 8
MFO = F // 128           # 24

CH = 512
NCH = S // CH            # 4

AS = 8.0
S_MIX = 4096.0
S_GU = 4096.0
S_UP = 4.0
S_O = 8192.0
ZETA = AS * S_UP         # 32


def build_program():
    nc = bacc.Bacc("TRN2", target_bir_lowering=False, debug=False)

    xT_d = nc.dram_tensor("xT", [128, NCH, KD, CH], F16, kind="ExternalInput").ap()
    wmix_d = nc.dram_tensor("w_mix", [128, 3 * MD, KD, 128], F8, kind="ExternalInput").ap()
    bmix_d = nc.dram_tensor("b_mix", [128, 3 * MD], F32, kind="ExternalInput").ap()
    wgu_d = nc.dram_tensor("w_gu", [MFO // 2, 128, 2, 2, KD, 128], F8, kind="ExternalInput").ap()
    wout_d = nc.dram_tensor("w_out", [128, MD, MFO, 128], F8, kind="ExternalInput").ap()
    out_d = nc.dram_tensor("outT", [MD, 128, S], F16, kind="ExternalOutput").ap()

    with tile.TileContext(nc) as tc, ExitStack() as top:
        # ---------- persistent tiles ----------
        cpool = top.enter_context(tc.tile_pool(name="consts", bufs=1))
        ones2 = cpool.tile([128, 2, 16], F8)
        nc.vector.memset(ones2[:], 1.0)
        bmix = cpool.tile([128, 3 * MD], F32)
        eps1 = cpool.tile([1, 1], F32)
        nc.vector.memset(eps1[:], EPS / (AS * AS))

        wmp = top.enter_context(tc.tile_pool(name="wmix", bufs=1))
        wmix = wmp.tile([128, 3 * MD, KD, 128], F8)
        wop = top.enter_context(tc.tile_pool(name="wout", bufs=1))
        wout = wop.tile([128, MD, MFO, 128], F8)

        p_wgu = top.enter_context(tc.tile_pool(name="wgu", bufs=4))
        p_xT = top.enter_context(tc.tile_pool(name="xT", bufs=2))
        p_x8 = top.enter_context(tc.tile_pool(name="x8", bufs=2))
        p_sq = top.enter_context(tc.tile_pool(name="sq", bufs=2))
        p_row = top.enter_context(tc.tile_pool(name="rows", bufs=4))
        p_bc = top.enter_context(tc.tile_pool(name="bc", bufs=3))
        p_sg = top.enter_context(tc.tile_pool(name="sg", bufs=2))
        p_sm = top.enter_context(tc.tile_pool(name="sm", bufs=4))
        p_xs = top.enter_context(tc.tile_pool(name="xs", bufs=3))
        p_at = top.enter_context(tc.tile_pool(name="at", bufs=3))
        p_hT = top.enter_context(tc.tile_pool(name="hT", bufs=2))
        p_o1 = top.enter_context(tc.tile_pool(name="o1", bufs=2))
        p_o1n = top.enter_context(tc.tile_pool(name="o1n", bufs=2))
        p_z = top.enter_context(tc.tile_pool(name="z", bufs=1))
        p_gt = top.enter_context(tc.tile_pool(name="gt", bufs=2))
        p_oT = top.enter_context(tc.tile_pool(name="oT", bufs=3))

        ps_mm = top.enter_context(tc.tile_pool(name="mm_ps", bufs=2, space="PSUM"))
        ps_ss = top.enter_context(tc.tile_pool(name="ss_ps", bufs=1, space="PSUM"))
        ps_gu = top.enter_context(tc.tile_pool(name="gu_ps", bufs=3, space="PSUM"))
        ps_y = top.enter_context(tc.tile_pool(name="y_ps", bufs=2, space="PSUM"))

        st = {}  # c -> dict of live tiles

        def stA_dma(c):
            d = st.setdefault(c, {})
            xT = p_xT.tile([128, KD, CH], F16, tag="xT", name=f"xT{c}")
            nc.sync.dma_start(xT[:], xT_d[:, c])
            d["xT"] = xT

        def _squares(c, src, key):
            """DVE squares (fp8) into a per-norm big tile."""
            d = st[c]
            sq = p_sq.tile([128, KD, CH], F8, tag="sq", name=f"sq_{key}_{c}")
            for j in range(KD // 2):
                nc.vector.tensor_tensor(sq[:, 2 * j:2 * j + 2],
                                        src[:, 2 * j:2 * j + 2],
                                        src[:, 2 * j:2 * j + 2], OP.mult)
            d[f"sq_{key}"] = sq

        def _reduce(c, key):
            """DR ones-reduce of the squares into a [1,CH] PSUM row."""
            d = st[c]
            sq = d[f"sq_{key}"]
            ss = ps_ss.tile([1, CH], F32, tag="ss", name=f"ss_{key}_{c}")
            for j in range(KD // 2):
                nc.tensor.matmul(ss[:], ones2[:, :, 0:1],
                                 sq[:, 2 * j:2 * j + 2],
                                 start=(j == 0), stop=(j == KD // 2 - 1),
                                 perf_mode=DR)
            d[f"ss_{key}"] = ss

        def norm_sqrt(c, key):
            d = st[c]
            srow = p_row.tile([1, CH], F32, tag="srow", name=f"srow_{key}_{c}")
            nc.scalar.activation(srow[:], d[f"ss_{key}"][:], AF.Sqrt,
                                 bias=eps1[:], scale=1.0 / (AS * AS * D))
            d[f"srow_{key}"] = srow

        def norm_rest(c, key, src, pool, dt, tag):
            """recip + f16 copy + partition broadcast + batched scale."""
            d = st[c]
            rrow = p_row.tile([1, CH], F32, tag="rrow", name=f"rrow_{key}_{c}")
            nc.vector.reciprocal_approx_fast(rrow[:], d[f"srow_{key}"][:])
            rrow16 = p_row.tile([1, CH], F16, tag="rrow16", name=f"r16_{key}_{c}")
            nc.vector.tensor_copy(rrow16[:], rrow[:])
            bc = p_bc.tile([128, CH], F16, tag="bc", name=f"bc_{key}_{c}")
            nc.gpsimd.partition_broadcast(bc[:], rrow16[:])
            outt = pool.tile([128, KD, CH], dt, tag=tag, name=f"{tag}{c}")
            nc.vector.tensor_tensor(
                outt[:], src[:], bc[:, None, :].broadcast_to((128, KD, CH)),
                OP.mult)
            d[tag] = outt

        def _proj(d, mt, out_ap, fn, sc=1.0):
            ps = ps_mm.tile([128, CH], F32, tag="mm", name=f"mm_{mt}")
            for j in range(KD // 2):
                nc.tensor.matmul(ps[:], wmix[:, mt, 2 * j:2 * j + 2, :],
                                 d["xnT"][:, 2 * j:2 * j + 2, :],
                                 start=(j == 0), stop=(j == KD // 2 - 1),
                                 perf_mode=DR)
            nc.scalar.activation(out_ap, ps[:], fn,
                                 bias=bmix[:, mt:mt + 1], scale=sc / (AS * S_MIX))

        def stC(c):
            """projections, activations, scan, residual (all f16)."""
            d = st[c]
            sg = p_sg.tile([128, KD, CH], F16, tag="sg", name=f"sg{c}")
            for kt in range(KD):          # g-pass: sg = tanh(g/2) = 2*sigma(g)-1
                _proj(d, kt, sg[:, kt], AF.Tanh, sc=0.5)
            ats = []
            for kt in range(KD):          # d-pass (sigmoid) + affine
                sd = p_sm.tile([128, CH], F16, tag="sd", name=f"sd{c}_{kt}")
                _proj(d, 2 * MD + kt, sd[:], AF.Tanh, sc=0.5)
                at = p_at.tile([128, CH], F16, tag="at", name=f"at{c}_{kt}")
                nc.vector.tensor_scalar(at[:], sd[:], 0.499, 0.5,
                                        OP.mult, OP.add)
                ats.append(at)
            hT = p_hT.tile([128, KD, CH], F16, tag="hT", name=f"hT{c}")
            hprev = st.get("hT_prev")
            for kt in range(KD):          # v-pass: tv = tanh(v)
                tv = p_sm.tile([128, CH], F16, tag="tv", name=f"tv{c}_{kt}")
                _proj(d, MD + kt, tv[:], AF.Tanh)
                # xs_eff = (sg+1)*tv = 2*sigma(g)*tanh(v); scan carries 2h
                xs = p_xs.tile([128, CH], F16, tag="xs", name=f"xs{c}_{kt}")
                nc.vector.scalar_tensor_tensor(xs[:], sg[:, kt], 1.0, tv[:],
                                               OP.add, OP.mult)
                init = 0.0 if c == 0 else hprev[:, kt, CH - 1:CH]
                nc.vector.tensor_tensor_scan(hT[:, kt], ats[kt][:], xs[:],
                                             init, OP.mult, OP.add)
            out1 = p_o1.tile([128, KD, CH], F16, tag="o1", name=f"o1_{c}")
            nc.vector.scalar_tensor_tensor(out1[:], hT[:], 0.5, d["xT"][:],
                                           OP.mult, OP.add)
            st["hT_prev"] = hT
            d["out1"] = out1

        def ffnA(c):
            """gate/up projections + z = silu(gate)*ups (z carries ZETA)."""
            d = st[c]
            z = p_z.tile([128, MFO, CH], F8, tag="z", name=f"z{c}")
            for mh in range(MFO // 2):    # paired weight loads
                wgu = p_wgu.tile([128, 2, 2, KD, 128], F8, tag="wgu")
                nc.sync.dma_start(wgu[:], wgu_d[mh])
                for i in range(2):
                    mg = 2 * mh + i
                    gps = ps_gu.tile([128, CH], F32, tag="gups", name=f"g{c}_{mg}")
                    for j in range(KD // 2):
                        nc.tensor.matmul(gps[:], wgu[:, i, 0, 2 * j:2 * j + 2, :],
                                         d["o1n"][:, 2 * j:2 * j + 2, :],
                                         start=(j == 0), stop=(j == KD // 2 - 1),
                                         perf_mode=DR)
                    ups = ps_gu.tile([128, CH], F32, tag="gups", name=f"u{c}_{mg}")
                    for j in range(KD // 2):
                        nc.tensor.matmul(ups[:], wgu[:, i, 1, 2 * j:2 * j + 2, :],
                                         d["o1n"][:, 2 * j:2 * j + 2, :],
                                         start=(j == 0), stop=(j == KD // 2 - 1),
                                         perf_mode=DR)
                    gate = p_gt.tile([128, CH], F16, tag="gate")
                    nc.scalar.activation(gate[:], gps[:], AF.Silu,
                                         bias=0.0, scale=1.0 / (AS * S_GU))
                    nc.vector.tensor_tensor(z[:, mg], gate[:], ups[:], OP.mult)
            d["z"] = z

        def ffnB(c):
            """W_out matmuls + residual + store (T layout, f16)."""
            d = st[c]
            s0 = c * CH
            for mo in range(MD):
                yps = ps_y.tile([128, CH], F32, tag="yps", name=f"y{c}_{mo}")
                for j in range(MFO // 2):
                    nc.tensor.matmul(yps[:], wout[:, mo, 2 * j:2 * j + 2, :],
                                     d["z"][:, 2 * j:2 * j + 2, :],
                                     start=(j == 0), stop=(j == MFO // 2 - 1),
                                     perf_mode=DR)
                oT = p_oT.tile([128, CH], F16, tag="oT")
                nc.vector.scalar_tensor_tensor(
                    oT[:], yps[:], 1.0 / (ZETA * S_O), d["out1"][:, mo],
                    OP.mult, OP.add)
                nc.scalar.dma_start(out_d[mo, :, s0:s0 + CH], oT[:])
            del st[c]

        # ---------- emission: fused software pipeline ----------
        # startup: x chunk 0 first; wmix/wout issued from the ACT hwdge queue
        # so the Sync queue stays clear for the xT/wgu streams
        stA_dma(0)
        for sec in range(3):
            nc.scalar.dma_start(wmix[:, sec * MD:(sec + 1) * MD],
                                wmix_d[:, sec * MD:(sec + 1) * MD])
        nc.scalar.dma_start(bmix[:], bmix_d[:])
        nc.scalar.dma_start(wout[:], wout_d[:])
        _squares(0, st[0]["xT"], "n1")
        _reduce(0, "n1")
        norm_sqrt(0, "n1")
        norm_rest(0, "n1", st[0]["xT"], p_x8, F8, "xnT")
        stA_dma(1)
        _squares(1, st[1]["xT"], "n1")
        _reduce(1, "n1")
        norm_sqrt(1, "n1")
        norm_rest(1, "n1", st[1]["xT"], p_x8, F8, "xnT")
        stC(0)
        _squares(0, st[0]["out1"], "n2")
        _reduce(0, "n2")
        norm_sqrt(0, "n2")
        norm_rest(0, "n2", st[0]["out1"], p_o1n, F8, "o1n")

        for c in range(NCH):
            if c + 2 < NCH:
                stA_dma(c + 2)
            if c + 1 < NCH:
                stC(c + 1)
                _squares(c + 1, st[c + 1]["out1"], "n2")
            if c + 2 < NCH:
                _squares(c + 2, st[c + 2]["xT"], "n1")
            ffnA(c)
            if c + 1 < NCH:
                _reduce(c + 1, "n2")
                norm_sqrt(c + 1, "n2")
            if c + 2 < NCH:
                _reduce(c + 2, "n1")
                norm_sqrt(c + 2, "n1")
            if c + 1 < NCH:
                norm_rest(c + 1, "n2", st[c + 1]["out1"], p_o1n, F8, "o1n")
            if c + 2 < NCH:
                norm_rest(c + 2, "n1", st[c + 2]["xT"], p_x8, F8, "xnT")
            ffnB(c)

    nc.compile()
    return nc


_NC = None


def _get_nc():
    global _NC
    if _NC is None:
        _NC = build_program()
    return _NC


def _q8(a, s):
    return np.clip(np.asarray(a, np.float32) * s, -240.0, 240.0).astype(
        ml_dtypes.float8_e4m3)


def _prep_weights(inputs):
    w1 = np.asarray(inputs["rms_mix_w"], np.float32)
    w2 = np.asarray(inputs["rms_ffn_w"], np.float32)
    Wg = np.asarray(inputs["Wg"], np.float32) * w1[None, :]
    Wv = np.asarray(inputs["Wv"], np.float32) * w1[None, :]
    Wd = np.asarray(inputs["Wd"], np.float32) * w1[None, :]
    Wcat = np.concatenate([Wg, Wv, Wd], axis=0)            # [3D, D]
    w_mix = _q8(np.ascontiguousarray(
        Wcat.T.reshape(KD, 128, 3 * MD, 128).transpose(1, 2, 0, 3)), S_MIX)
    bcat = np.concatenate([0.5 * np.asarray(inputs["bg"], np.float32),
                           np.asarray(inputs["bv"], np.float32),
                           0.5 * np.asarray(inputs["bd"], np.float32)])
    b_mix = np.ascontiguousarray(bcat.reshape(3 * MD, 128).T).astype(np.float32)
    Wgate = np.asarray(inputs["W_gate"], np.float32) * w2[None, :]
    Wup = np.asarray(inputs["W_up"], np.float32) * w2[None, :]
    # [MFO, 2(g|u), 128(k-part), KD, 128(out)]
    Wg8 = _q8(np.ascontiguousarray(
        (Wgate * S_GU).T.reshape(KD, 128, MFO, 128).transpose(2, 1, 0, 3)), 1.0)
    Wu8 = _q8(np.ascontiguousarray(
        (Wup * S_UP).T.reshape(KD, 128, MFO, 128).transpose(2, 1, 0, 3)), 1.0)
    w_gu = np.ascontiguousarray(
        np.stack([Wg8, Wu8], axis=1)             # [MFO, 2, 128, KD, 128]
        .reshape(MFO // 2, 2, 2, 128, KD, 128)
        .transpose(0, 3, 1, 2, 4, 5))            # [MFO/2, 128, 2, 2, KD, 128]
    WoT = np.asarray(inputs["W_out"], np.float32).T        # [F, D]
    w_out = _q8(np.ascontiguousarray(
        WoT.reshape(MFO, 128, MD, 128).transpose(1, 2, 0, 3)), S_O)
    return {"w_mix": w_mix, "b_mix": b_mix, "w_gu": w_gu, "w_out": w_out}


def run(inputs, trace=False, **kw):
    x = np.asarray(inputs["x"], np.float32)
    shared = _prep_weights(inputs)
    in_maps = []
    for b in range(B):
        xT = np.ascontiguousarray(
            x[b].T.astype(np.float16).reshape(KD, 128, NCH, CH)
            .transpose(1, 2, 0, 3))
        in_maps.append(dict(shared, xT=xT))
    res = run_bass_kernel_spmd(_get_nc(), in_maps, list(range(B)), trace=trace, **kw)
    out = np.stack([
        np.asarray(res.results[b]["outT"], np.float16)
        .astype(np.float32).reshape(D, S).T
        for b in range(B)])
    return np.ascontiguousarray(out), res


def kernel(**inputs) -> np.ndarray:
    out, _ = run(inputs)
    return out


if __name__ == "__main__":
    d = np.load("/root/problem/ref.npz")
    inputs = {k: d[k] for k in d.files if k != "expected"}
    out, _ = run(inputs)
    exp = d["expected"]
    err = np.linalg.norm((out - exp).ravel()) / np.linalg.norm(exp.ravel())
    print("rel err", err)


# revision 17
# speedup vs baseline: 1.2333x; 1.2333x over previous
"""MinGRU block kernel v4 for Trainium2 (Bass/Tile), SPMD over 8 NeuronCores.

Problem: B=8, S=2048, D=1024, F=3072. Data-parallel: one batch row per core.

Host-side transposes (x arrives as f16 xT [KD,128,S], output leaves as f16
outT [MD,128,S]); fused per-chunk software pipeline; out1 residual resident in
SBUF f16; norm squares on DVE + DoubleRow ones-reduce on PE; per-engine
instruction streams are emission-ordered to avoid ACT table thrash (sigma /
tanh / silu / sqrt grouped; 4 table switches per chunk) and to keep the PE
matmul stream dense (reduce-MMs placed after the FFN gate/up block so the
DVE scan chain is long since done).

Scales (as v2): acts fp8 with AS=8 folded into rms reciprocal; mixer weights
x S_MIX, gate x S_GU, up x S_UP, out x S_O; z carries ZETA = AS*S_UP = 32;
final readout descales by 1/(ZETA*S_O).
"""

import os
import sys
from contextlib import ExitStack

import numpy as np
import ml_dtypes

for _p in ("/opt/trn_rl_repo", "/root/.axon_site/_ro/trn_rl_repo"):
    if os.path.isdir(_p) and _p not in sys.path:
        sys.path.insert(0, _p)

import concourse.bass as bass
import concourse.tile as tile
from concourse import bacc, mybir
from concourse.bass_utils import run_bass_kernel_spmd

F32 = mybir.dt.float32
F16 = mybir.dt.float16
F8 = mybir.dt.float8e4
AF = mybir.ActivationFunctionType
OP = mybir.AluOpType
DR = mybir.MatmulPerfMode.DoubleRow

B, S, D, F = 8, 2048, 1024, 3072
EPS = 1e-6
KD = D // 128            # 8
MD = D // 128            # 8
MFO = F // 128           # 24

CH = 512
NCH = S // CH            # 4

AS = 8.0
S_MIX = 4096.0
S_GU = 4096.0
S_UP = 4.0
S_O = 8192.0
ZETA = AS * S_UP         # 32


def build_program():
    nc = bacc.Bacc("TRN2", target_bir_lowering=False, debug=False)

    xT_d = nc.dram_tensor("xT", [128, NCH, KD, CH], F16, kind="ExternalInput").ap()
    wmix_d = nc.dram_tensor("w_mix", [128, 3 * MD, KD, 128], F8, kind="ExternalInput").ap()
    bmix_d = nc.dram_tensor("b_mix", [128, 3 * MD], F32, kind="ExternalInput").ap()
    wgu_d = nc.dram_tensor("w_gu", [MFO // 2, 128, 2, 2, KD, 128], F8, kind="ExternalInput").ap()
    wout_d = nc.dram_tensor("w_out", [128, MD, MFO, 128], F8, kind="ExternalInput").ap()
    out_d = nc.dram_tensor("outT", [MD, 128, S], F16, kind="ExternalOutput").ap()

    with tile.TileContext(nc) as tc, ExitStack() as top:
        # ---------- persistent tiles ----------
        cpool = top.enter_context(tc.tile_pool(name="consts", bufs=1))
        ones2 = cpool.tile([128, 2, 16], F8)
        nc.vector.memset(ones2[:], 1.0)
        bmix = cpool.tile([128, 3 * MD], F32)
        eps1 = cpool.tile([1, 1], F32)
        nc.vector.memset(eps1[:], EPS / (AS * AS))

        wmp = top.enter_context(tc.tile_pool(name="wmix", bufs=1))
        wmix = wmp.tile([128, 3 * MD, KD, 128], F8)
        wop = top.enter_context(tc.tile_pool(name="wout", bufs=1))
        wout = wop.tile([128, MD, MFO, 128], F8)

        p_wgu = top.enter_context(tc.tile_pool(name="wgu", bufs=4))
        p_xT = top.enter_context(tc.tile_pool(name="xT", bufs=2))
        p_x8 = top.enter_context(tc.tile_pool(name="x8", bufs=2))
        p_sq = top.enter_context(tc.tile_pool(name="sq", bufs=2))
        p_row = top.enter_context(tc.tile_pool(name="rows", bufs=4))
        p_bc = top.enter_context(tc.tile_pool(name="bc", bufs=3))
        p_sg = top.enter_context(tc.tile_pool(name="sg", bufs=2))
        p_sm = top.enter_context(tc.tile_pool(name="sm", bufs=4))
        p_xs = top.enter_context(tc.tile_pool(name="xs", bufs=3))
        p_at = top.enter_context(tc.tile_pool(name="at", bufs=3))
        p_hT = top.enter_context(tc.tile_pool(name="hT", bufs=2))
        p_o1 = top.enter_context(tc.tile_pool(name="o1", bufs=2))
        p_o1n = top.enter_context(tc.tile_pool(name="o1n", bufs=2))
        p_z = top.enter_context(tc.tile_pool(name="z", bufs=1))
        p_gt = top.enter_context(tc.tile_pool(name="gt", bufs=2))
        p_oT = top.enter_context(tc.tile_pool(name="oT", bufs=3))

        ps_mm = top.enter_context(tc.tile_pool(name="mm_ps", bufs=2, space="PSUM"))
        ps_ss = top.enter_context(tc.tile_pool(name="ss_ps", bufs=1, space="PSUM"))
        ps_gu = top.enter_context(tc.tile_pool(name="gu_ps", bufs=3, space="PSUM"))
        ps_y = top.enter_context(tc.tile_pool(name="y_ps", bufs=2, space="PSUM"))

        st = {}  # c -> dict of live tiles

        def stA_dma(c):
            d = st.setdefault(c, {})
            xT = p_xT.tile([128, KD, CH], F16, tag="xT", name=f"xT{c}")
            nc.sync.dma_start(xT[:], xT_d[:, c])
            d["xT"] = xT

        def _sq_reduce(c, src, key):
            """DVE squares (fp8) + DR ones-reduce into a [1,CH] PSUM row."""
            d = st[c]
            ss = ps_ss.tile([1, CH], F32, tag="ss", name=f"ss_{key}_{c}")
            for j in range(KD // 2):
                sq = p_sq.tile([128, 2, CH], F8, tag="sq", name=f"sq_{key}_{c}_{j}")
                nc.vector.tensor_tensor(sq[:], src[:, 2 * j:2 * j + 2],
                                        src[:, 2 * j:2 * j + 2], OP.mult)
                nc.tensor.matmul(ss[:], ones2[:, :, 0:1], sq[:],
                                 start=(j == 0), stop=(j == KD // 2 - 1),
                                 perf_mode=DR)
            d[f"ss_{key}"] = ss

        def norm_sqrt(c, key):
            d = st[c]
            srow = p_row.tile([1, CH], F32, tag="srow", name=f"srow_{key}_{c}")
            nc.scalar.activation(srow[:], d[f"ss_{key}"][:], AF.Sqrt,
                                 bias=eps1[:], scale=1.0 / (AS * AS * D))
            d[f"srow_{key}"] = srow

        def norm_rest(c, key, src, pool, dt, tag):
            """recip + f16 copy + partition broadcast + batched scale."""
            d = st[c]
            rrow = p_row.tile([1, CH], F32, tag="rrow", name=f"rrow_{key}_{c}")
            nc.vector.reciprocal_approx_fast(rrow[:], d[f"srow_{key}"][:])
            rrow16 = p_row.tile([1, CH], F16, tag="rrow16", name=f"r16_{key}_{c}")
            nc.vector.tensor_copy(rrow16[:], rrow[:])
            bc = p_bc.tile([128, CH], F16, tag="bc", name=f"bc_{key}_{c}")
            nc.gpsimd.partition_broadcast(bc[:], rrow16[:])
            outt = pool.tile([128, KD, CH], dt, tag=tag, name=f"{tag}{c}")
            nc.vector.tensor_tensor(
                outt[:], src[:], bc[:, None, :].broadcast_to((128, KD, CH)),
                OP.mult)
            d[tag] = outt

        def _proj(d, mt, out_ap, fn, sc=1.0):
            ps = ps_mm.tile([128, CH], F32, tag="mm", name=f"mm_{mt}")
            for j in range(KD // 2):
                nc.tensor.matmul(ps[:], wmix[:, mt, 2 * j:2 * j + 2, :],
                                 d["xnT"][:, 2 * j:2 * j + 2, :],
                                 start=(j == 0), stop=(j == KD // 2 - 1),
                                 perf_mode=DR)
            nc.scalar.activation(out_ap, ps[:], fn,
                                 bias=bmix[:, mt:mt + 1], scale=sc / (AS * S_MIX))

        def stC(c):
            """projections, activations, scan, residual (all f16)."""
            d = st[c]
            sg = p_sg.tile([128, KD, CH], F16, tag="sg", name=f"sg{c}")
            for kt in range(KD):          # g-pass: sg = tanh(g/2) = 2*sigma(g)-1
                _proj(d, kt, sg[:, kt], AF.Tanh, sc=0.5)
            ats = []
            for kt in range(KD):          # d-pass (sigmoid) + affine
                sd = p_sm.tile([128, CH], F16, tag="sd", name=f"sd{c}_{kt}")
                _proj(d, 2 * MD + kt, sd[:], AF.Tanh, sc=0.5)
                at = p_at.tile([128, CH], F16, tag="at", name=f"at{c}_{kt}")
                nc.vector.tensor_scalar(at[:], sd[:], 0.499, 0.5,
                                        OP.mult, OP.add)
                ats.append(at)
            hT = p_hT.tile([128, KD, CH], F16, tag="hT", name=f"hT{c}")
            hprev = st.get("hT_prev")
            for kt in range(KD):          # v-pass: tv = tanh(v)
                tv = p_sm.tile([128, CH], F16, tag="tv", name=f"tv{c}_{kt}")
                _proj(d, MD + kt, tv[:], AF.Tanh)
                # xs_eff = (sg+1)*tv = 2*sigma(g)*tanh(v); scan carries 2h
                xs = p_xs.tile([128, CH], F16, tag="xs", name=f"xs{c}_{kt}")
                nc.vector.scalar_tensor_tensor(xs[:], sg[:, kt], 1.0, tv[:],
                                               OP.add, OP.mult)
                init = 0.0 if c == 0 else hprev[:, kt, CH - 1:CH]
                nc.vector.tensor_tensor_scan(hT[:, kt], ats[kt][:], xs[:],
                                             init, OP.mult, OP.add)
            out1 = p_o1.tile([128, KD, CH], F16, tag="o1", name=f"o1_{c}")
            nc.vector.scalar_tensor_tensor(out1[:], hT[:], 0.5, d["xT"][:],
                                           OP.mult, OP.add)
            st["hT_prev"] = hT
            d["out1"] = out1

        def ffnA(c):
            """gate/up projections + z = silu(gate)*ups (z carries ZETA)."""
            d = st[c]
            z = p_z.tile([128, MFO, CH], F8, tag="z", name=f"z{c}")
            for mh in range(MFO // 2):    # paired weight loads
                wgu = p_wgu.tile([128, 2, 2, KD, 128], F8, tag="wgu")
                nc.sync.dma_start(wgu[:], wgu_d[mh])
                for i in range(2):
                    mg = 2 * mh + i
                    gps = ps_gu.tile([128, CH], F32, tag="gups", name=f"g{c}_{mg}")
                    for j in range(KD // 2):
                        nc.tensor.matmul(gps[:], wgu[:, i, 0, 2 * j:2 * j + 2, :],
                                         d["o1n"][:, 2 * j:2 * j + 2, :],
                                         start=(j == 0), stop=(j == KD // 2 - 1),
                                         perf_mode=DR)
                    ups = ps_gu.tile([128, CH], F32, tag="gups", name=f"u{c}_{mg}")
                    for j in range(KD // 2):
                        nc.tensor.matmul(ups[:], wgu[:, i, 1, 2 * j:2 * j + 2, :],
                                         d["o1n"][:, 2 * j:2 * j + 2, :],
                                         start=(j == 0), stop=(j == KD // 2 - 1),
                                         perf_mode=DR)
                    gate = p_gt.tile([128, CH], F16, tag="gate")
                    nc.scalar.activation(gate[:], gps[:], AF.Silu,
                                         bias=0.0, scale=1.0 / (AS * S_GU))
                    nc.vector.tensor_tensor(z[:, mg], gate[:], ups[:], OP.mult)
            d["z"] = z

        def ffnB(c):
            """W_out matmuls + residual + store (T layout, f16)."""
            d = st[c]
            s0 = c * CH
            for mo in range(MD):
                yps = ps_y.tile([128, CH], F32, tag="yps", name=f"y{c}_{mo}")
                for j in range(MFO // 2):
                    nc.tensor.matmul(yps[:], wout[:, mo, 2 * j:2 * j + 2, :],
                                     d["z"][:, 2 * j:2 * j + 2, :],
                                     start=(j == 0), stop=(j == MFO // 2 - 1),
                                     perf_mode=DR)
                oT = p_oT.tile([128, CH], F16, tag="oT")
                nc.vector.scalar_tensor_tensor(
                    oT[:], yps[:], 1.0 / (ZETA * S_O), d["out1"][:, mo],
                    OP.mult, OP.add)
                nc.scalar.dma_start(out_d[mo, :, s0:s0 + CH], oT[:])
            del st[c]

        # ---------- emission: fused software pipeline ----------
        # startup: x chunk 0 first; wmix/wout issued from the ACT hwdge queue
        # so the Sync queue stays clear for the xT/wgu streams
        stA_dma(0)
        for sec in range(3):
            nc.scalar.dma_start(wmix[:, sec * MD:(sec + 1) * MD],
                                wmix_d[:, sec * MD:(sec + 1) * MD])
        nc.scalar.dma_start(bmix[:], bmix_d[:])
        nc.scalar.dma_start(wout[:], wout_d[:])
        _sq_reduce(0, st[0]["xT"], "n1")
        norm_sqrt(0, "n1")
        norm_rest(0, "n1", st[0]["xT"], p_x8, F8, "xnT")
        stA_dma(1)
        _sq_reduce(1, st[1]["xT"], "n1")
        norm_sqrt(1, "n1")
        norm_rest(1, "n1", st[1]["xT"], p_x8, F8, "xnT")
        stC(0)
        _sq_reduce(0, st[0]["out1"], "n2")
        norm_sqrt(0, "n2")
        norm_rest(0, "n2", st[0]["out1"], p_o1n, F8, "o1n")

        for c in range(NCH):
            if c + 1 < NCH:
                stC(c + 1)
            ffnA(c)
            if c + 2 < NCH:
                stA_dma(c + 2)
            if c + 1 < NCH:
                _sq_reduce(c + 1, st[c + 1]["out1"], "n2")
                norm_sqrt(c + 1, "n2")
            if c + 2 < NCH:
                _sq_reduce(c + 2, st[c + 2]["xT"], "n1")
                norm_sqrt(c + 2, "n1")
            if c + 1 < NCH:
                norm_rest(c + 1, "n2", st[c + 1]["out1"], p_o1n, F8, "o1n")
            if c + 2 < NCH:
                norm_rest(c + 2, "n1", st[c + 2]["xT"], p_x8, F8, "xnT")
            ffnB(c)

    nc.compile()
    return nc


_NC = None


def _get_nc():
    global _NC
    if _NC is None:
        _NC = build_program()
    return _NC


def _q8(a, s):
    return np.clip(np.asarray(a, np.float32) * s, -240.0, 240.0).astype(
        ml_dtypes.float8_e4m3)


def _prep_weights(inputs):
    w1 = np.asarray(inputs["rms_mix_w"], np.float32)
    w2 = np.asarray(inputs["rms_ffn_w"], np.float32)
    Wg = np.asarray(inputs["Wg"], np.float32) * w1[None, :]
    Wv = np.asarray(inputs["Wv"], np.float32) * w1[None, :]
    Wd = np.asarray(inputs["Wd"], np.float32) * w1[None, :]
    Wcat = np.concatenate([Wg, Wv, Wd], axis=0)            # [3D, D]
    w_mix = _q8(np.ascontiguousarray(
        Wcat.T.reshape(KD, 128, 3 * MD, 128).transpose(1, 2, 0, 3)), S_MIX)
    bcat = np.concatenate([0.5 * np.asarray(inputs["bg"], np.float32),
                           np.asarray(inputs["bv"], np.float32),
                           0.5 * np.asarray(inputs["bd"], np.float32)])
    b_mix = np.ascontiguousarray(bcat.reshape(3 * MD, 128).T).astype(np.float32)
    Wgate = np.asarray(inputs["W_gate"], np.float32) * w2[None, :]
    Wup = np.asarray(inputs["W_up"], np.float32) * w2[None, :]
    # [MFO, 2(g|u), 128(k-part), KD, 128(out)]
    Wg8 = _q8(np.ascontiguousarray(
        (Wgate * S_GU).T.reshape(KD, 128, MFO, 128).transpose(2, 1, 0, 3)), 1.0)
    Wu8 = _q8(np.ascontiguousarray(
        (Wup * S_UP).T.reshape(KD, 128, MFO, 128).transpose(2, 1, 0, 3)), 1.0)
    w_gu = np.ascontiguousarray(
        np.stack([Wg8, Wu8], axis=1)             # [MFO, 2, 128, KD, 128]
        .reshape(MFO // 2, 2, 2, 128, KD, 128)
        .transpose(0, 3, 1, 2, 4, 5))            # [MFO/2, 128, 2, 2, KD, 128]
    WoT = np.asarray(inputs["W_out"], np.float32).T        # [F, D]
    w_out = _q8(np.ascontiguousarray(
        WoT.reshape(MFO, 128, MD, 128).transpose(1, 2, 0, 3)), S_O)
    return {"w_mix": w_mix, "b_mix": b_mix, "w_gu": w_gu, "w_out": w_out}


def run(inputs, trace=False, **kw):
    x = np.asarray(inputs["x"], np.float32)
    shared = _prep_weights(inputs)
    in_maps = []
    for b in range(B):
        xT = np.ascontiguousarray(
            x[b].T.astype(np.float16).reshape(KD, 128, NCH, CH)
            .transpose(1, 2, 0, 3))
        in_maps.append(dict(shared, xT=xT))
    res = run_bass_kernel_spmd(_get_nc(), in_maps, list(range(B)), trace=trace, **kw)
    out = np.stack([
        np.asarray(res.results[b]["outT"], np.float16)
        .astype(np.float32).reshape(D, S).T
        for b in range(B)])
    return np.ascontiguousarray(out), res


def kernel(**inputs) -> np.ndarray:
    out, _ = run(inputs)
    return out


if __name__ == "__main__":
    d = np.load("/root/problem/ref.npz")
    inputs = {k: d[k] for k in d.files if k != "expected"}
    out, _ = run(inputs)
    exp = d["expected"]
    err = np.linalg.norm((out - exp).ravel()) / np.linalg.norm(exp.ravel())
    print("rel err", err)


# revision 19
# speedup vs baseline: 1.2356x; 1.0019x over previous
"""MinGRU block kernel v4 for Trainium2 (Bass/Tile), SPMD over 8 NeuronCores.

Problem: B=8, S=2048, D=1024, F=3072. Data-parallel: one batch row per core.

Host-side transposes (x arrives as f16 xT [KD,128,S], output leaves as f16
outT [MD,128,S]); fused per-chunk software pipeline; out1 residual resident in
SBUF f16; norm squares on DVE + DoubleRow ones-reduce on PE; per-engine
instruction streams are emission-ordered to avoid ACT table thrash (sigma /
tanh / silu / sqrt grouped; 4 table switches per chunk) and to keep the PE
matmul stream dense (reduce-MMs placed after the FFN gate/up block so the
DVE scan chain is long since done).

Scales (as v2): acts fp8 with AS=8 folded into rms reciprocal; mixer weights
x S_MIX, gate x S_GU, up x S_UP, out x S_O; z carries ZETA = AS*S_UP = 32;
final readout descales by 1/(ZETA*S_O).
"""

import os
import sys
from contextlib import ExitStack

import numpy as np
import ml_dtypes

for _p in ("/opt/trn_rl_repo", "/root/.axon_site/_ro/trn_rl_repo"):
    if os.path.isdir(_p) and _p not in sys.path:
        sys.path.insert(0, _p)

import concourse.bass as bass
import concourse.tile as tile
from concourse import bacc, mybir
from concourse.bass_utils import run_bass_kernel_spmd

F32 = mybir.dt.float32
F16 = mybir.dt.float16
F8 = mybir.dt.float8e4
AF = mybir.ActivationFunctionType
OP = mybir.AluOpType
DR = mybir.MatmulPerfMode.DoubleRow

B, S, D, F = 8, 2048, 1024, 3072
EPS = 1e-6
KD = D // 128            # 8
MD = D // 128            # 8
MFO = F // 128           # 24

CH = 512
NCH = S // CH            # 4

AS = 8.0
S_MIX = 4096.0
S_GU = 4096.0
S_UP = 4.0
S_O = 8192.0
ZETA = AS * S_UP         # 32


def build_program():
    nc = bacc.Bacc("TRN2", target_bir_lowering=False, debug=False)

    xT_d = nc.dram_tensor("xT", [128, NCH, KD, CH], F16, kind="ExternalInput").ap()
    wmix_d = nc.dram_tensor("w_mix", [128, 3 * MD, KD, 128], F8, kind="ExternalInput").ap()
    bmix_d = nc.dram_tensor("b_mix", [128, 3 * MD], F32, kind="ExternalInput").ap()
    wgu_d = nc.dram_tensor("w_gu", [MFO // 2, 128, 2, 2, KD, 128], F8, kind="ExternalInput").ap()
    wout_d = nc.dram_tensor("w_out", [128, MD, MFO, 128], F8, kind="ExternalInput").ap()
    out_d = nc.dram_tensor("outT", [MD, 128, S], F16, kind="ExternalOutput").ap()

    with tile.TileContext(nc) as tc, ExitStack() as top:
        # ---------- persistent tiles ----------
        cpool = top.enter_context(tc.tile_pool(name="consts", bufs=1))
        ones2 = cpool.tile([128, 2, 16], F8)
        nc.vector.memset(ones2[:], 1.0)
        bmix = cpool.tile([128, 3 * MD], F32)
        eps1 = cpool.tile([1, 1], F32)
        nc.vector.memset(eps1[:], EPS / (AS * AS))

        wmp = top.enter_context(tc.tile_pool(name="wmix", bufs=1))
        wmix = wmp.tile([128, 3 * MD, KD, 128], F8)
        wop = top.enter_context(tc.tile_pool(name="wout", bufs=1))
        wout = wop.tile([128, MD, MFO, 128], F8)

        p_wgu = top.enter_context(tc.tile_pool(name="wgu", bufs=3))
        p_xT = top.enter_context(tc.tile_pool(name="xT", bufs=2))
        p_x8 = top.enter_context(tc.tile_pool(name="x8", bufs=2))
        p_sq = top.enter_context(tc.tile_pool(name="sq", bufs=2))
        p_row = top.enter_context(tc.tile_pool(name="rows", bufs=4))
        p_bc = top.enter_context(tc.tile_pool(name="bc", bufs=3))
        p_sg = top.enter_context(tc.tile_pool(name="sg", bufs=2))
        p_sm = top.enter_context(tc.tile_pool(name="sm", bufs=4))
        p_xs = top.enter_context(tc.tile_pool(name="xs", bufs=3))
        p_at = top.enter_context(tc.tile_pool(name="at", bufs=3))
        p_hT = top.enter_context(tc.tile_pool(name="hT", bufs=2))
        p_o1 = top.enter_context(tc.tile_pool(name="o1", bufs=2))
        p_o1n = top.enter_context(tc.tile_pool(name="o1n", bufs=2))
        p_z = top.enter_context(tc.tile_pool(name="z", bufs=1))
        p_gt = top.enter_context(tc.tile_pool(name="gt", bufs=2))
        p_oT = top.enter_context(tc.tile_pool(name="oT", bufs=3))

        ps_mm = top.enter_context(tc.tile_pool(name="mm_ps", bufs=2, space="PSUM"))
        ps_ss = top.enter_context(tc.tile_pool(name="ss_ps", bufs=1, space="PSUM"))
        ps_gu = top.enter_context(tc.tile_pool(name="gu_ps", bufs=3, space="PSUM"))
        ps_y = top.enter_context(tc.tile_pool(name="y_ps", bufs=2, space="PSUM"))

        st = {}  # c -> dict of live tiles

        def stA_dma(c):
            d = st.setdefault(c, {})
            xT = p_xT.tile([128, KD, CH], F16, tag="xT", name=f"xT{c}")
            nc.sync.dma_start(xT[:], xT_d[:, c])
            d["xT"] = xT

        def _sq_reduce(c, src, key):
            """DVE squares (fp8) + DR ones-reduce into a [1,CH] PSUM row."""
            d = st[c]
            ss = ps_ss.tile([1, CH], F32, tag="ss", name=f"ss_{key}_{c}")
            for j in range(KD // 2):
                sq = p_sq.tile([128, 2, CH], F8, tag="sq", name=f"sq_{key}_{c}_{j}")
                nc.vector.tensor_tensor(sq[:], src[:, 2 * j:2 * j + 2],
                                        src[:, 2 * j:2 * j + 2], OP.mult)
                nc.tensor.matmul(ss[:], ones2[:, :, 0:1], sq[:],
                                 start=(j == 0), stop=(j == KD // 2 - 1),
                                 perf_mode=DR)
            d[f"ss_{key}"] = ss

        def _squares_n1(c):
            """DVE squares for norm1 (xT-based) into a big tile (early)."""
            d = st[c]
            sq = p_sq.tile([128, KD, CH], F8, tag="sqb", name=f"sqb_{c}")
            xT = d["xT"]
            for j in range(KD // 2):
                nc.vector.tensor_tensor(sq[:, 2 * j:2 * j + 2],
                                        xT[:, 2 * j:2 * j + 2],
                                        xT[:, 2 * j:2 * j + 2], OP.mult)
            d["sq_n1"] = sq

        def _reduce_n1(c):
            d = st[c]
            sq = d["sq_n1"]
            ss = ps_ss.tile([1, CH], F32, tag="ss", name=f"ss_n1_{c}")
            for j in range(KD // 2):
                nc.tensor.matmul(ss[:], ones2[:, :, 0:1],
                                 sq[:, 2 * j:2 * j + 2],
                                 start=(j == 0), stop=(j == KD // 2 - 1),
                                 perf_mode=DR)
            d["ss_n1"] = ss

        def norm_sqrt(c, key):
            d = st[c]
            srow = p_row.tile([1, CH], F32, tag="srow", name=f"srow_{key}_{c}")
            nc.scalar.activation(srow[:], d[f"ss_{key}"][:], AF.Sqrt,
                                 bias=eps1[:], scale=1.0 / (AS * AS * D))
            d[f"srow_{key}"] = srow

        def norm_rest(c, key, src, pool, dt, tag):
            """recip + f16 copy + partition broadcast + batched scale."""
            d = st[c]
            rrow = p_row.tile([1, CH], F32, tag="rrow", name=f"rrow_{key}_{c}")
            nc.vector.reciprocal_approx_fast(rrow[:], d[f"srow_{key}"][:])
            rrow16 = p_row.tile([1, CH], F16, tag="rrow16", name=f"r16_{key}_{c}")
            nc.vector.tensor_copy(rrow16[:], rrow[:])
            bc = p_bc.tile([128, CH], F16, tag="bc", name=f"bc_{key}_{c}")
            nc.gpsimd.partition_broadcast(bc[:], rrow16[:])
            outt = pool.tile([128, KD, CH], dt, tag=tag, name=f"{tag}{c}")
            nc.vector.tensor_tensor(
                outt[:], src[:], bc[:, None, :].broadcast_to((128, KD, CH)),
                OP.mult)
            d[tag] = outt

        def _proj(d, mt, out_ap, fn, sc=1.0):
            ps = ps_mm.tile([128, CH], F32, tag="mm", name=f"mm_{mt}")
            for j in range(KD // 2):
                nc.tensor.matmul(ps[:], wmix[:, mt, 2 * j:2 * j + 2, :],
                                 d["xnT"][:, 2 * j:2 * j + 2, :],
                                 start=(j == 0), stop=(j == KD // 2 - 1),
                                 perf_mode=DR)
            nc.scalar.activation(out_ap, ps[:], fn,
                                 bias=bmix[:, mt:mt + 1], scale=sc / (AS * S_MIX))

        def stC(c):
            """projections, activations, scan, residual (all f16)."""
            d = st[c]
            sg = p_sg.tile([128, KD, CH], F16, tag="sg", name=f"sg{c}")
            for kt in range(KD):          # g-pass: sg = tanh(g/2) = 2*sigma(g)-1
                _proj(d, kt, sg[:, kt], AF.Tanh, sc=0.5)
            ats = []
            for kt in range(KD):          # d-pass (sigmoid) + affine
                sd = p_sm.tile([128, CH], F16, tag="sd", name=f"sd{c}_{kt}")
                _proj(d, 2 * MD + kt, sd[:], AF.Tanh, sc=0.5)
                at = p_at.tile([128, CH], F16, tag="at", name=f"at{c}_{kt}")
                nc.vector.tensor_scalar(at[:], sd[:], 0.499, 0.5,
                                        OP.mult, OP.add)
                ats.append(at)
            hT = p_hT.tile([128, KD, CH], F16, tag="hT", name=f"hT{c}")
            hprev = st.get("hT_prev")
            for kt in range(KD):          # v-pass: tv = tanh(v)
                tv = p_sm.tile([128, CH], F16, tag="tv", name=f"tv{c}_{kt}")
                _proj(d, MD + kt, tv[:], AF.Tanh)
                # xs_eff = (sg+1)*tv = 2*sigma(g)*tanh(v); scan carries 2h
                xs = p_xs.tile([128, CH], F16, tag="xs", name=f"xs{c}_{kt}")
                nc.vector.scalar_tensor_tensor(xs[:], sg[:, kt], 1.0, tv[:],
                                               OP.add, OP.mult)
                init = 0.0 if c == 0 else hprev[:, kt, CH - 1:CH]
                nc.vector.tensor_tensor_scan(hT[:, kt], ats[kt][:], xs[:],
                                             init, OP.mult, OP.add)
            out1 = p_o1.tile([128, KD, CH], F16, tag="o1", name=f"o1_{c}")
            nc.vector.scalar_tensor_tensor(out1[:], hT[:], 0.5, d["xT"][:],
                                           OP.mult, OP.add)
            st["hT_prev"] = hT
            d["out1"] = out1

        def ffnA(c):
            """gate/up projections + z = silu(gate)*ups (z carries ZETA)."""
            d = st[c]
            z = p_z.tile([128, MFO, CH], F8, tag="z", name=f"z{c}")
            for mh in range(MFO // 2):    # paired weight loads
                wgu = p_wgu.tile([128, 2, 2, KD, 128], F8, tag="wgu")
                nc.sync.dma_start(wgu[:], wgu_d[mh])
                for i in range(2):
                    mg = 2 * mh + i
                    gps = ps_gu.tile([128, CH], F32, tag="gups", name=f"g{c}_{mg}")
                    for j in range(KD // 2):
                        nc.tensor.matmul(gps[:], wgu[:, i, 0, 2 * j:2 * j + 2, :],
                                         d["o1n"][:, 2 * j:2 * j + 2, :],
                                         start=(j == 0), stop=(j == KD // 2 - 1),
                                         perf_mode=DR)
                    ups = ps_gu.tile([128, CH], F32, tag="gups", name=f"u{c}_{mg}")
                    for j in range(KD // 2):
                        nc.tensor.matmul(ups[:], wgu[:, i, 1, 2 * j:2 * j + 2, :],
                                         d["o1n"][:, 2 * j:2 * j + 2, :],
                                         start=(j == 0), stop=(j == KD // 2 - 1),
                                         perf_mode=DR)
                    gate = p_gt.tile([128, CH], F16, tag="gate")
                    nc.scalar.activation(gate[:], gps[:], AF.Silu,
                                         bias=0.0, scale=1.0 / (AS * S_GU))
                    nc.vector.tensor_tensor(z[:, mg], gate[:], ups[:], OP.mult)
            d["z"] = z

        def ffnB(c):
            """W_out matmuls + residual + store (T layout, f16)."""
            d = st[c]
            s0 = c * CH
            for mo in range(MD):
                yps = ps_y.tile([128, CH], F32, tag="yps", name=f"y{c}_{mo}")
                for j in range(MFO // 2):
                    nc.tensor.matmul(yps[:], wout[:, mo, 2 * j:2 * j + 2, :],
                                     d["z"][:, 2 * j:2 * j + 2, :],
                                     start=(j == 0), stop=(j == MFO // 2 - 1),
                                     perf_mode=DR)
                oT = p_oT.tile([128, CH], F16, tag="oT")
                nc.vector.scalar_tensor_tensor(
                    oT[:], yps[:], 1.0 / (ZETA * S_O), d["out1"][:, mo],
                    OP.mult, OP.add)
                nc.scalar.dma_start(out_d[mo, :, s0:s0 + CH], oT[:])
            del st[c]

        # ---------- emission: fused software pipeline ----------
        # startup: x chunk 0 first; wmix/wout issued from the ACT hwdge queue
        # so the Sync queue stays clear for the xT/wgu streams
        stA_dma(0)
        for sec in range(3):
            nc.scalar.dma_start(wmix[:, sec * MD:(sec + 1) * MD],
                                wmix_d[:, sec * MD:(sec + 1) * MD])
        nc.scalar.dma_start(bmix[:], bmix_d[:])
        nc.scalar.dma_start(wout[:], wout_d[:])
        _sq_reduce(0, st[0]["xT"], "n1")
        norm_sqrt(0, "n1")
        norm_rest(0, "n1", st[0]["xT"], p_x8, F8, "xnT")
        stA_dma(1)
        _sq_reduce(1, st[1]["xT"], "n1")
        norm_sqrt(1, "n1")
        norm_rest(1, "n1", st[1]["xT"], p_x8, F8, "xnT")
        stC(0)
        _sq_reduce(0, st[0]["out1"], "n2")
        norm_sqrt(0, "n2")
        norm_rest(0, "n2", st[0]["out1"], p_o1n, F8, "o1n")

        for c in range(NCH):
            if c + 2 < NCH:
                stA_dma(c + 2)
            if c + 1 < NCH:
                stC(c + 1)
            if c + 2 < NCH:
                _squares_n1(c + 2)
            ffnA(c)
            if c + 1 < NCH:
                _sq_reduce(c + 1, st[c + 1]["out1"], "n2")
                norm_sqrt(c + 1, "n2")
            if c + 2 < NCH:
                _reduce_n1(c + 2)
                norm_sqrt(c + 2, "n1")
            if c + 1 < NCH:
                norm_rest(c + 1, "n2", st[c + 1]["out1"], p_o1n, F8, "o1n")
            if c + 2 < NCH:
                norm_rest(c + 2, "n1", st[c + 2]["xT"], p_x8, F8, "xnT")
            ffnB(c)

    nc.compile()
    return nc


_NC = None


def _get_nc():
    global _NC
    if _NC is None:
        _NC = build_program()
    return _NC


def _q8(a, s):
    return np.clip(np.asarray(a, np.float32) * s, -240.0, 240.0).astype(
        ml_dtypes.float8_e4m3)


def _prep_weights(inputs):
    w1 = np.asarray(inputs["rms_mix_w"], np.float32)
    w2 = np.asarray(inputs["rms_ffn_w"], np.float32)
    Wg = np.asarray(inputs["Wg"], np.float32) * w1[None, :]
    Wv = np.asarray(inputs["Wv"], np.float32) * w1[None, :]
    Wd = np.asarray(inputs["Wd"], np.float32) * w1[None, :]
    Wcat = np.concatenate([Wg, Wv, Wd], axis=0)            # [3D, D]
    w_mix = _q8(np.ascontiguousarray(
        Wcat.T.reshape(KD, 128, 3 * MD, 128).transpose(1, 2, 0, 3)), S_MIX)
    bcat = np.concatenate([0.5 * np.asarray(inputs["bg"], np.float32),
                           np.asarray(inputs["bv"], np.float32),
                           0.5 * np.asarray(inputs["bd"], np.float32)])
    b_mix = np.ascontiguousarray(bcat.reshape(3 * MD, 128).T).astype(np.float32)
    Wgate = np.asarray(inputs["W_gate"], np.float32) * w2[None, :]
    Wup = np.asarray(inputs["W_up"], np.float32) * w2[None, :]
    # [MFO, 2(g|u), 128(k-part), KD, 128(out)]
    Wg8 = _q8(np.ascontiguousarray(
        (Wgate * S_GU).T.reshape(KD, 128, MFO, 128).transpose(2, 1, 0, 3)), 1.0)
    Wu8 = _q8(np.ascontiguousarray(
        (Wup * S_UP).T.reshape(KD, 128, MFO, 128).transpose(2, 1, 0, 3)), 1.0)
    w_gu = np.ascontiguousarray(
        np.stack([Wg8, Wu8], axis=1)             # [MFO, 2, 128, KD, 128]
        .reshape(MFO // 2, 2, 2, 128, KD, 128)
        .transpose(0, 3, 1, 2, 4, 5))            # [MFO/2, 128, 2, 2, KD, 128]
    WoT = np.asarray(inputs["W_out"], np.float32).T        # [F, D]
    w_out = _q8(np.ascontiguousarray(
        WoT.reshape(MFO, 128, MD, 128).transpose(1, 2, 0, 3)), S_O)
    return {"w_mix": w_mix, "b_mix": b_mix, "w_gu": w_gu, "w_out": w_out}


def run(inputs, trace=False, **kw):
    x = np.asarray(inputs["x"], np.float32)
    shared = _prep_weights(inputs)
    in_maps = []
    for b in range(B):
        xT = np.ascontiguousarray(
            x[b].T.astype(np.float16).reshape(KD, 128, NCH, CH)
            .transpose(1, 2, 0, 3))
        in_maps.append(dict(shared, xT=xT))
    res = run_bass_kernel_spmd(_get_nc(), in_maps, list(range(B)), trace=trace, **kw)
    out = np.stack([
        np.asarray(res.results[b]["outT"], np.float16)
        .astype(np.float32).reshape(D, S).T
        for b in range(B)])
    return np.ascontiguousarray(out), res


def kernel(**inputs) -> np.ndarray:
    out, _ = run(inputs)
    return out


if __name__ == "__main__":
    d = np.load("/root/problem/ref.npz")
    inputs = {k: d[k] for k in d.files if k != "expected"}
    out, _ = run(inputs)
    exp = d["expected"]
    err = np.linalg.norm((out - exp).ravel()) / np.linalg.norm(exp.ravel())
    print("rel err", err)
